# revision 7
# baseline (speedup 1.0000x reference)
"""DCRNN (PEMS-BAY) Trainium2 Bass kernel, data-parallel over batch on 8 cores.

Layouts per core (local batch BL=8):
  A-layout: [feature partitions, b*384 + n]  (n padded 325->384; 8*384 = 3072 cols)
  B-layout: [node-chunk partitions (128/128/69), b*Fout + f]
gconv (W-first):  out = X@A0 + S @ (X@W1 + S @ (X@(2*W2)))
  P2,P1 = W-matmuls in A-layout; transpose down to B; X1 = S@P2B; Q = X1+P1B;
  R = S@QB; PE-transposes of R accumulate onto the X@A0 PSUM banks; activation
  evacuates PSUM -> SBUF.
State tile XH per layer: rows 0:64 = h, rows 64:128 = x (padded features).
"""
import sys
import os
import numpy as np

sys.path.insert(0, "/opt/trn_rl_repo")

import concourse.bass as bass  # noqa: E402
import concourse.mybir as mybir  # noqa: E402
import concourse.tile as tile  # noqa: E402
from concourse import bacc  # noqa: E402
from concourse.bass_utils import run_bass_kernel_spmd  # noqa: E402
from concourse.masks import make_identity  # noqa: E402

# problem constants
N = 325
B = 64
T = 12
HZ = 12
U = 64
DIN = 2
DOUT = 1
NCORES = 8
BL = B // NCORES          # 8 local batch
NB = 384                  # padded node stride per batch
AF = BL * NB              # 3072 A-layout free width
NCH = [(0, 128), (128, 128), (256, 69)]   # node chunks (offset, len)
NBANK = AF // 512         # 6 psum banks for a full A row

F32 = mybir.dt.float32
MMDT = mybir.dt.float32   # matmul input dtype (float32 | float32r)
AFT = mybir.ActivationFunctionType

CELLS = ["enc0", "enc1", "dec0", "dec1"]
CELL_DIN = {"enc0": DIN, "enc1": U, "dec0": DOUT, "dec1": U}

_BUILD_CACHE = {}
LAST_RESULT = None


def _install_ntff_hook():
    """Register the axon NTFF profiling hook if the image lacks antenv.axon_hooks."""
    import types
    import antenv
    if getattr(antenv, "axon_hooks", None) is not None:
        return
    m = types.ModuleType("antenv.axon_hooks")
    state = {"h": None}
    m.set_axon_ntff_profile_hook = lambda h: state.__setitem__("h", h)
    m.get_axon_ntff_profile_hook = lambda: state["h"]
    sys.modules["antenv.axon_hooks"] = m
    antenv.axon_hooks = m
    try:
        from trn_agent_boot.trn_boot import _ntff_profile_via_ctypes
        hook = _ntff_profile_via_ctypes("/opt/axon/libaxon_pjrt.so")
        if hook is not None:
            m.set_axon_ntff_profile_hook(hook)
    except Exception:
        pass


def _pad_w(w, din, fout):
    """(3F, fout) -> three [128, fout] padded mats A0, W1, 2*W2.

    Padded row map: rows 0:64 <- h/rh features (orig rows din:F),
    rows 64:64+din <- x features (orig rows 0:din). Others zero.
    """
    f = din + U
    w0, w1, w2 = w[0:f], w[f:2 * f], w[2 * f:3 * f]

    def pad(m):
        p = np.zeros((128, fout), np.float32)
        p[0:64] = m[din:f]
        p[64:64 + din] = m[0:din]
        return p

    return pad(w0 - w2), pad(w1), pad(2.0 * w2)


def _build(nsteps_enc, nsteps_dec):
    key = (nsteps_enc, nsteps_dec)
    if key in _BUILD_CACHE:
        return _BUILD_CACHE[key]

    nc = bacc.Bacc()
    # ---- DRAM params ----
    x_in = nc.declare_dram_parameter("x", [T, DIN, AF], F32, isOutput=False)
    s_in = nc.declare_dram_parameter("s", [N, N], F32, isOutput=False)
    wparams = {}
    for c in CELLS:
        for nm, shp in [("gA0", [128, 128]), ("gW1", [128, 128]),
                        ("gW2", [128, 128]), ("cA0", [128, 64]),
                        ("cW1", [128, 64]), ("cW2", [128, 64]),
                        ("gb", [128, 1]), ("cb", [64, 1])]:
            wparams[f"{c}_{nm}"] = nc.declare_dram_parameter(
                f"{c}_{nm}", shp, F32, isOutput=False)
    wparams["pW"] = nc.declare_dram_parameter("pW", [64, 1], F32, isOutput=False)
    wparams["pb"] = nc.declare_dram_parameter("pb", [1, 1], F32, isOutput=False)
    out_d = nc.declare_dram_parameter("out", [HZ, 1, BL, N], F32, isOutput=True)

    with tile.TileContext(nc) as tc:
        with tc.tile_pool(name="const", bufs=1) as cp, \
             tc.tile_pool(name="state", bufs=1) as st, \
             tc.tile_pool(name="pa", bufs=1) as pa, \
             tc.tile_pool(name="bp", bufs=1) as bp, \
             tc.tile_pool(name="preactp", bufs=1, space="PSUM") as prp, \
             tc.tile_pool(name="pstagep", bufs=2, space="PSUM") as psp:

            # ---- constants to SBUF ----
            wt = {}
            for c in CELLS:
                for nm in ["gA0", "gW1", "gW2"]:
                    wt[f"{c}_{nm}"] = cp.tile([128, 128], MMDT, tag=f"{c}_{nm}", name=f"{c}_{nm}")
                for nm in ["cA0", "cW1", "cW2"]:
                    wt[f"{c}_{nm}"] = cp.tile([128, 64], MMDT, tag=f"{c}_{nm}", name=f"{c}_{nm}")
                wt[f"{c}_gb"] = cp.tile([128, 1], F32, tag=f"{c}_gb", name=f"{c}_gb")
                wt[f"{c}_cb"] = cp.tile([64, 1], F32, tag=f"{c}_cb", name=f"{c}_cb")
            wt["pW"] = cp.tile([64, 1], MMDT, tag="pW", name="pW")
            wt["pb"] = cp.tile([1, 1], F32, tag="pb", name="pb")
            for k, t in wt.items():
                nc.sync.dma_start(out=t, in_=wparams[k][:])
            s_t = []
            for ci, (c0, cl) in enumerate(NCH):
                stl = cp.tile([128, N], MMDT, tag=f"s{ci}", name=f"s{ci}")
                nc.sync.dma_start(out=stl[0:cl, :], in_=s_in[c0:c0 + cl, :])
                s_t.append(stl)
            ident = cp.tile([128, 128], F32, tag="ident")
            make_identity(nc, ident)

            # ---- state tiles ----
            xh = {c: st.tile([128, AF], MMDT, tag=f"xh_{c}", name=f"xh_{c}") for c in CELLS}
            xr = st.tile([128, AF], MMDT, tag="xr")
            r_t = st.tile([64, AF], F32, tag="r_t")
            u_t = st.tile([64, AF], F32, tag="u_t")
            c_t = st.tile([64, AF], F32, tag="c_t")
            t1 = st.tile([64, AF], F32, tag="t1")
            t2 = st.tile([64, AF], F32, tag="t2")
            for t in list(xh.values()) + [xr]:
                nc.vector.memset(t, 0.0)
            tc.strict_bb_all_engine_barrier()

            def gconv(cell, parts, wprefix, fout, bank_of):
                """Emit one gconv. parts: list of (wslice_fn, rhs_ap, tpos).
                Returns preact psum tile [128, AF] (rows 0:fout valid,
                includes bias NOT yet); caller evacuates with activation."""
                a0 = wt[f"{cell}_{wprefix}A0"]
                w1 = wt[f"{cell}_{wprefix}W1"]
                w2 = wt[f"{cell}_{wprefix}W2"]
                fh = BL * fout            # B-layout free width
                nhalf = fh // 512

                p2a = pa.tile([128, AF], MMDT, tag="p2a")
                p1a = pa.tile([128, AF], MMDT, tag="p1a")
                # P2, P1 W-matmuls -> psum chunk -> evac to SBUF (ACT)
                for w_, dst in ((w2, p2a), (w1, p1a)):
                    for ci in range(NBANK):
                        sl = slice(ci * 512, (ci + 1) * 512)
                        pt = psp.tile([128, 512], F32, tag="pstage")
                        for pi, (wsl, rhs, tpos) in enumerate(parts):
                            nc.tensor.matmul(
                                pt[0:fout, :], wsl(w_), rhs[:, sl],
                                start=(pi == 0), stop=(pi == len(parts) - 1),
                                tile_position=tpos)
                        nc.scalar.copy(dst[0:fout, sl], pt[0:fout, :])

                # P0 -> preact (start groups, keep open for up-transposes)
                preact = prp.tile([128, AF], F32, tag="preact")
                for ci in range(NBANK):
                    sl = slice(ci * 512, (ci + 1) * 512)
                    for pi, (wsl, rhs, tpos) in enumerate(parts):
                        nc.tensor.matmul(
                            preact[0:fout, sl], wsl(a0), rhs[:, sl],
                            start=(pi == 0), stop=False, tile_position=tpos)

                # down-transposes P2A,P1A -> P2B,P1B
                gsz = 512 // fout         # 4 (fout=128) or 8 (fout=64)
                bt = {}
                for role, src in (("p2b", p2a), ("p1b", p1a)):
                    tiles = [bp.tile([128, fh], F32, tag=f"{role}{ci}", name=f"{role}{ci}")
                             for ci in range(3)]
                    for ci, (c0, cl) in enumerate(NCH):
                        for g in range(BL // gsz):
                            dt = psp.tile([128, 512], F32, tag="pstage")
                            for j in range(gsz):
                                b = g * gsz + j
                                nc.tensor.matmul(
                                    dt[0:cl, j * fout:(j + 1) * fout],
                                    src[0:fout, b * NB + c0: b * NB + c0 + cl],
                                    ident[0:fout, 0:fout],
                                    is_transpose=True,
                                    start=(j == 0), stop=(j == gsz - 1))
                            nc.vector.tensor_copy(
                                tiles[ci][0:cl, g * 512:(g + 1) * 512],
                                dt[0:cl, :])
                    bt[role] = tiles

                # X1 = S@P2B ; Q = X1 + P1B ; R = S@QB -> RB
                qb = [bp.tile([128, fh], MMDT, tag=f"qb{ci}", name=f"qb{ci}") for ci in range(3)]
                rbt = [bp.tile([128, fh], F32, tag=f"rb{ci}", name=f"rb{ci}") for ci in range(3)]
                for dst, srcs, srcadd in ((qb, bt["p2b"], bt["p1b"]),
                                          (rbt, qb, None)):
                    for mi, (m0, ml) in enumerate(NCH):
                        for h in range(nhalf):
                            hs = slice(h * 512, (h + 1) * 512)
                            xt = psp.tile([128, 512], F32, tag="pstage")
                            for ki, (k0, kl) in enumerate(NCH):
                                nc.tensor.matmul(
                                    xt[0:ml, :], s_t[ki][0:kl, m0:m0 + ml],
                                    srcs[ki][0:kl, hs],
                                    start=(ki == 0), stop=(ki == 2))
                            if srcadd is not None:
                                nc.vector.tensor_tensor(
                                    dst[mi][0:ml, hs], xt[0:ml, :],
                                    srcadd[mi][0:ml, hs], mybir.AluOpType.add)
                            else:
                                nc.vector.tensor_copy(dst[mi][0:ml, hs],
                                                      xt[0:ml, :])

                # up-transposes RB -> preact (accumulate); stop on last per bank
                nwr = [0] * NBANK
                order = [(b, ci) for b in range(BL) for ci in range(3)]
                for b, ci in order:
                    nwr[bank_of(b, ci)] += 1
                seen = [0] * NBANK
                for b, ci in order:
                    c0, cl = NCH[ci]
                    bk = bank_of(b, ci)
                    seen[bk] += 1
                    nc.tensor.matmul(
                        preact[0:fout, b * NB + c0: b * NB + c0 + cl],
                        rbt[ci][0:cl, b * fout:(b + 1) * fout],
                        ident[0:cl, 0:cl],
                        is_transpose=True,
                        start=False, stop=(seen[bk] == nwr[bk]))
                return preact

            def bank_of(b, ci):
                return (b * NB + NCH[ci][0]) // 512

            def cell(cname, xh_t, xh_next):
                """One DCGRU cell. x in xh_t[64:128], h in xh_t[0:64].
                Writes h' to xh_t[0:64] and (if xh_next) xh_next[64:128]."""
                din = CELL_DIN[cname]
                gparts = [(lambda w: w[0:128, :], xh_t[0:128, :], None)]
                pre_g = gconv(cname, gparts, "g", 128, bank_of)
                gb = wt[f"{cname}_gb"]
                for ci in range(NBANK):
                    sl = slice(ci * 512, (ci + 1) * 512)
                    nc.scalar.activation(r_t[:, sl], pre_g[0:64, sl],
                                         AFT.Sigmoid, bias=gb[0:64, 0:1])
                    nc.scalar.activation(u_t[:, sl], pre_g[64:128, sl],
                                         AFT.Sigmoid, bias=gb[64:128, 0:1])
                # xr: rows 0:64 = r*h, rows 64:64+din = x (copy)
                nc.vector.tensor_tensor(xr[0:64, :], r_t[:, :], xh_t[0:64, :],
                                        mybir.AluOpType.mult)
                nc.vector.tensor_copy(xr[64:64 + din, :],
                                      xh_t[64:64 + din, :])
                cparts = [(lambda w: w[0:128, :], xr[0:128, :], None)]
                pre_c = gconv(cname, cparts, "c", 64, bank_of)
                cb = wt[f"{cname}_cb"]
                for ci in range(NBANK):
                    sl = slice(ci * 512, (ci + 1) * 512)
                    nc.scalar.activation(c_t[:, sl], pre_c[0:64, sl],
                                         AFT.Tanh, bias=cb[0:64, 0:1])
                # h' = c + u*(h-c)
                nc.vector.tensor_tensor(t1[:, :], xh_t[0:64, :], c_t[:, :],
                                        mybir.AluOpType.subtract)
                nc.vector.tensor_tensor(t2[:, :], u_t[:, :], t1[:, :],
                                        mybir.AluOpType.mult)
                nc.vector.tensor_tensor(xh_t[0:64, :], c_t[:, :], t2[:, :],
                                        mybir.AluOpType.add)
                if xh_next is not None:
                    nc.vector.tensor_copy(xh_next[64:128, :], xh_t[0:64, :])

            # ---- encoder ----
            for t in range(nsteps_enc):
                nc.sync.dma_start(out=xh["enc0"][64:66, :], in_=x_in[t])
                cell("enc0", xh["enc0"], xh["enc1"])
                cell("enc1", xh["enc1"], None)

            # ---- copy encoder state to decoder ----
            nc.vector.tensor_copy(xh["dec0"][0:64, :], xh["enc0"][0:64, :])
            nc.vector.tensor_copy(xh["dec1"][0:64, :], xh["enc1"][0:64, :])

            # ---- decoder ----
            for t in range(nsteps_dec):
                cell("dec0", xh["dec0"], xh["dec1"])
                cell("dec1", xh["dec1"], None)
                # projection: out = hd1' @ pW + pb -> xh_dec0 row 64 + DRAM
                for ci in range(NBANK):
                    sl = slice(ci * 512, (ci + 1) * 512)
                    pt = psp.tile([128, 512], F32, tag="pstage")
                    nc.tensor.matmul(pt[0:1, :], wt["pW"][0:64, :],
                                     xh["dec1"][0:64, sl],
                                     start=True, stop=True)
                    nc.scalar.activation(xh["dec0"][64:65, sl], pt[0:1, :],
                                         AFT.Identity, bias=wt["pb"][0:1, 0:1])
                ov = xh["dec0"][64:65, :].rearrange("p (b n) -> p b n", b=BL)
                nc.sync.dma_start(out=out_d[t], in_=ov[:, :, 0:N])

    nc.finalize()
    _BUILD_CACHE[key] = nc
    return nc


def _prep_inputs(inputs, support, weights):
    """Host-side prep. Returns (shared_map, per_core_x list)."""
    shared = {"s": np.ascontiguousarray(support, np.float32)}
    for c in CELLS:
        din = CELL_DIN[c]
        ga0, gw1, gw2 = _pad_w(weights[f"{c}_gate_W"], din, 2 * U)
        ca0, cw1, cw2 = _pad_w(weights[f"{c}_cand_W"], din, U)
        gb = np.zeros((128, 1), np.float32)
        gb[:, 0] = weights[f"{c}_gate_b"]
        cb = np.zeros((64, 1), np.float32)
        cb[:, 0] = weights[f"{c}_cand_b"]
        shared.update({f"{c}_gA0": ga0, f"{c}_gW1": gw1, f"{c}_gW2": gw2,
                       f"{c}_cA0": ca0, f"{c}_cW1": cw1, f"{c}_cW2": cw2,
                       f"{c}_gb": gb, f"{c}_cb": cb})
    shared["pW"] = np.ascontiguousarray(weights["proj_W"], np.float32)
    shared["pb"] = np.asarray(weights["proj_b"], np.float32).reshape(1, 1)

    # inputs (T, B, N*DIN) -> per-core (T, DIN, AF) with node padding
    x = np.asarray(inputs, np.float32).reshape(T, B, N, DIN)
    per_core = []
    for c in range(NCORES):
        xc = x[:, c * BL:(c + 1) * BL]                  # (T, BL, N, DIN)
        xp = np.zeros((T, DIN, BL, NB), np.float32)
        xp[:, :, :, 0:N] = xc.transpose(0, 3, 1, 2)
        per_core.append(xp.reshape(T, DIN, AF))
    return shared, per_core


def kernel(**inputs) -> np.ndarray:
    support = np.asarray(inputs["support"], np.float32)
    weights = {k: np.asarray(v, np.float32) for k, v in inputs.items()
               if k not in ("inputs", "support")}
    shared, per_core_x = _prep_inputs(inputs["inputs"], support, weights)

    nc = _build(T, HZ)
    if os.environ.get("DCRNN_TRACE"):
        _install_ntff_hook()
    in_maps = [dict(shared, x=per_core_x[c]) for c in range(NCORES)]
    res = run_bass_kernel_spmd(nc, in_maps, list(range(NCORES)),
                               trace=bool(os.environ.get("DCRNN_TRACE")))
    global LAST_RESULT
    LAST_RESULT = res
    if res.exec_time_ns is not None:
        print(f"HW exec time: {res.exec_time_ns} ns")
    outs = [res.results[c]["out"].reshape(HZ, BL, N) for c in range(NCORES)]
    return np.concatenate(outs, axis=1).astype(np.float32)


if __name__ == "__main__":
    sys.path.insert(0, "/root/problem")
    import reference
    ins = reference.setup_inputs()
    ins = {k: np.asarray(v) for k, v in ins.items()}
    exp = np.asarray(reference.reference(**ins))
    act = kernel(**ins)
    err = np.max(np.abs(act - exp)) / (np.abs(exp).max() + 1e-30)
    print("Relative error:", err)


# revision 11
# speedup vs baseline: 1.5864x; 1.5864x over previous
"""DCRNN (PEMS-BAY) Trainium2 Bass kernel, data-parallel over batch on 8 cores.

Layouts per core (local batch BL=8):
  A-layout: [feature partitions, b*384 + n]  (n padded 325->384; 8*384 = 3072 cols)
  B-layout: [node-chunk partitions (128/128/69), b*Fout + f]
gconv (W-first):  out = X@A0 + S @ (X@W1 + S @ (X@(2*W2)))
  P2,P1 = W-matmuls in A-layout; transpose down to B; X1 = S@P2B; Q = X1+P1B;
  R = S@QB; PE-transposes of R accumulate onto the X@A0 PSUM banks; activation
  evacuates PSUM -> SBUF.
State tile XH per layer: rows 0:64 = h, rows 64:128 = x (padded features).
"""
import sys
import os
import numpy as np

sys.path.insert(0, "/opt/trn_rl_repo")

import concourse.bass as bass  # noqa: E402
import concourse.mybir as mybir  # noqa: E402
import concourse.tile as tile  # noqa: E402
from concourse import bacc  # noqa: E402
from concourse.bass_utils import run_bass_kernel_spmd  # noqa: E402
from concourse.masks import make_identity  # noqa: E402

# problem constants
N = 325
B = 64
T = 12
HZ = 12
U = 64
DIN = 2
DOUT = 1
NCORES = 8
BL = B // NCORES          # 8 local batch
NB = 384                  # padded node stride per batch
AF = BL * NB              # 3072 A-layout free width
NCH = [(0, 128), (128, 128), (256, 69)]   # node chunks (offset, len)
NBANK = AF // 512         # 6 psum banks for a full A row

F32 = mybir.dt.float32
MMDT = mybir.dt.float32r  # matmul input dtype (float32 | float32r)
AFT = mybir.ActivationFunctionType

CELLS = ["enc0", "enc1", "dec0", "dec1"]
CELL_DIN = {"enc0": DIN, "enc1": U, "dec0": DOUT, "dec1": U}

_BUILD_CACHE = {}
LAST_RESULT = None


def _install_ntff_hook():
    """Register the axon NTFF profiling hook if the image lacks antenv.axon_hooks."""
    import types
    import antenv
    if getattr(antenv, "axon_hooks", None) is not None:
        return
    m = types.ModuleType("antenv.axon_hooks")
    state = {"h": None}
    m.set_axon_ntff_profile_hook = lambda h: state.__setitem__("h", h)
    m.get_axon_ntff_profile_hook = lambda: state["h"]
    sys.modules["antenv.axon_hooks"] = m
    antenv.axon_hooks = m
    try:
        from trn_agent_boot.trn_boot import _ntff_profile_via_ctypes
        hook = _ntff_profile_via_ctypes("/opt/axon/libaxon_pjrt.so")
        if hook is not None:
            m.set_axon_ntff_profile_hook(hook)
    except Exception:
        pass


def _pad_w(w, din, fout):
    """(3F, fout) -> three [128, fout] padded mats A0, W1, 2*W2.

    Padded row map: rows 0:64 <- h/rh features (orig rows din:F),
    rows 64:64+din <- x features (orig rows 0:din). Others zero.
    """
    f = din + U
    w0, w1, w2 = w[0:f], w[f:2 * f], w[2 * f:3 * f]

    def pad(m):
        p = np.zeros((128, fout), np.float32)
        p[0:64] = m[din:f]
        p[64:64 + din] = m[0:din]
        return p

    return pad(w0 - w2), pad(w1), pad(2.0 * w2)


def _build(nsteps_enc, nsteps_dec):
    key = (nsteps_enc, nsteps_dec)
    if key in _BUILD_CACHE:
        return _BUILD_CACHE[key]

    nc = bacc.Bacc()
    # ---- DRAM params ----
    x_in = nc.declare_dram_parameter("x", [T, DIN, AF], MMDT, isOutput=False)
    s_in = nc.declare_dram_parameter("s", [N, N], MMDT, isOutput=False)
    wparams = {}
    for c in CELLS:
        for nm, shp in [("gA0", [128, 128]), ("gW1", [128, 128]),
                        ("gW2", [128, 128]), ("cA0", [128, 64]),
                        ("cW1", [128, 64]), ("cW2", [128, 64]),
                        ("gb", [128, 1]), ("cb", [64, 1])]:
            dt_ = F32 if nm in ("gb", "cb") else MMDT
            wparams[f"{c}_{nm}"] = nc.declare_dram_parameter(
                f"{c}_{nm}", shp, dt_, isOutput=False)
    wparams["pW"] = nc.declare_dram_parameter("pW", [64, 1], MMDT, isOutput=False)
    wparams["pb"] = nc.declare_dram_parameter("pb", [1, 1], F32, isOutput=False)
    out_d = nc.declare_dram_parameter("out", [HZ, 1, BL, N], F32, isOutput=True)

    with tile.TileContext(nc) as tc:
        with tc.tile_pool(name="const", bufs=1) as cp, \
             tc.tile_pool(name="state", bufs=1) as st, \
             tc.tile_pool(name="pa", bufs=1) as pa, \
             tc.tile_pool(name="bp", bufs=1) as bp, \
             tc.tile_pool(name="preactp", bufs=1, space="PSUM") as prp, \
             tc.tile_pool(name="pstagep", bufs=2, space="PSUM") as psp:

            # ---- constants to SBUF ----
            wt = {}
            for c in CELLS:
                for nm in ["gA0", "gW1", "gW2"]:
                    wt[f"{c}_{nm}"] = cp.tile([128, 128], MMDT, tag=f"{c}_{nm}", name=f"{c}_{nm}")
                for nm in ["cA0", "cW1", "cW2"]:
                    wt[f"{c}_{nm}"] = cp.tile([128, 64], MMDT, tag=f"{c}_{nm}", name=f"{c}_{nm}")
                wt[f"{c}_gb"] = cp.tile([128, 1], F32, tag=f"{c}_gb", name=f"{c}_gb")
                wt[f"{c}_cb"] = cp.tile([64, 1], F32, tag=f"{c}_cb", name=f"{c}_cb")
            wt["pW"] = cp.tile([64, 1], MMDT, tag="pW", name="pW")
            wt["pb"] = cp.tile([1, 1], F32, tag="pb", name="pb")
            for k, t in wt.items():
                nc.sync.dma_start(out=t, in_=wparams[k][:])
            s_t = []
            for ci, (c0, cl) in enumerate(NCH):
                stl = cp.tile([128, N], MMDT, tag=f"s{ci}", name=f"s{ci}")
                nc.sync.dma_start(out=stl[0:cl, :], in_=s_in[c0:c0 + cl, :])
                s_t.append(stl)
            ident = cp.tile([128, 128], F32, tag="ident")
            make_identity(nc, ident)

            # ---- state tiles ----
            xh = {c: st.tile([128, AF], MMDT, tag=f"xh_{c}", name=f"xh_{c}") for c in CELLS}
            xr = st.tile([128, AF], MMDT, tag="xr")
            r_t = st.tile([64, AF], F32, tag="r_t")
            u_t = st.tile([64, AF], F32, tag="u_t")
            c_t = st.tile([64, AF], F32, tag="c_t")
            t1 = st.tile([64, AF], F32, tag="t1")
            t2 = st.tile([64, AF], F32, tag="t2")
            for t in list(xh.values()) + [xr]:
                nc.vector.memset(t[:, :].bitcast(F32), 0.0)
            tc.strict_bb_all_engine_barrier()

            def gconv(cell, parts, wprefix, fout, bank_of):
                """Emit one gconv. parts: list of (wslice_fn, rhs_ap, tpos).
                Returns preact psum tile [128, AF] (rows 0:fout valid,
                includes bias NOT yet); caller evacuates with activation."""
                a0 = wt[f"{cell}_{wprefix}A0"]
                w1 = wt[f"{cell}_{wprefix}W1"]
                w2 = wt[f"{cell}_{wprefix}W2"]
                fh = BL * fout            # B-layout free width
                nhalf = fh // 512

                p2a = pa.tile([128, AF], F32, tag="p2a")
                p1a = pa.tile([128, AF], F32, tag="p1a")
                # P2, P1 W-matmuls -> psum chunk -> evac to SBUF (ACT)
                for w_, dst in ((w2, p2a), (w1, p1a)):
                    for ci in range(NBANK):
                        sl = slice(ci * 512, (ci + 1) * 512)
                        pt = psp.tile([128, 512], F32, tag="pstage")
                        for pi, (wsl, rhs, tpos) in enumerate(parts):
                            nc.tensor.matmul(
                                pt[0:fout, :], wsl(w_), rhs[:, sl],
                                start=(pi == 0), stop=(pi == len(parts) - 1),
                                tile_position=tpos)
                        nc.scalar.copy(dst[0:fout, sl], pt[0:fout, :])

                # P0 -> preact (start groups, keep open for up-transposes)
                preact = prp.tile([128, AF], F32, tag="preact")
                for ci in range(NBANK):
                    sl = slice(ci * 512, (ci + 1) * 512)
                    for pi, (wsl, rhs, tpos) in enumerate(parts):
                        nc.tensor.matmul(
                            preact[0:fout, sl], wsl(a0), rhs[:, sl],
                            start=(pi == 0), stop=False, tile_position=tpos)

                # down-transposes P2A,P1A -> P2B,P1B
                gsz = 512 // fout         # 4 (fout=128) or 8 (fout=64)
                bt = {}
                for role, src in (("p2b", p2a), ("p1b", p1a)):
                    dt_ = MMDT if role == "p2b" else F32
                    tiles = [bp.tile([128, fh], dt_, tag=f"{role}{ci}", name=f"{role}{ci}")
                             for ci in range(3)]
                    for ci, (c0, cl) in enumerate(NCH):
                        for g in range(BL // gsz):
                            dt = psp.tile([128, 512], F32, tag="pstage")
                            for j in range(gsz):
                                b = g * gsz + j
                                nc.tensor.matmul(
                                    dt[0:cl, j * fout:(j + 1) * fout],
                                    src[0:fout, b * NB + c0: b * NB + c0 + cl],
                                    ident[0:fout, 0:fout],
                                    is_transpose=True,
                                    start=(j == 0), stop=(j == gsz - 1))
                            nc.vector.tensor_copy(
                                tiles[ci][0:cl, g * 512:(g + 1) * 512],
                                dt[0:cl, :])
                    bt[role] = tiles

                # X1 = S@P2B ; Q = X1 + P1B ; R = S@QB -> RB
                qb = [bp.tile([128, fh], MMDT, tag=f"qb{ci}", name=f"qb{ci}") for ci in range(3)]
                rbt = [bp.tile([128, fh], F32, tag=f"rb{ci}", name=f"rb{ci}") for ci in range(3)]
                for dst, srcs, srcadd in ((qb, bt["p2b"], bt["p1b"]),
                                          (rbt, qb, None)):
                    for mi, (m0, ml) in enumerate(NCH):
                        for h in range(nhalf):
                            hs = slice(h * 512, (h + 1) * 512)
                            xt = psp.tile([128, 512], F32, tag="pstage")
                            for ki, (k0, kl) in enumerate(NCH):
                                nc.tensor.matmul(
                                    xt[0:ml, :], s_t[ki][0:kl, m0:m0 + ml],
                                    srcs[ki][0:kl, hs],
                                    start=(ki == 0), stop=(ki == 2))
                            if srcadd is not None:
                                nc.vector.tensor_tensor(
                                    dst[mi][0:ml, hs], xt[0:ml, :],
                                    srcadd[mi][0:ml, hs], mybir.AluOpType.add)
                            else:
                                nc.vector.tensor_copy(dst[mi][0:ml, hs],
                                                      xt[0:ml, :])

                # up-transposes RB -> preact (accumulate); stop on last per bank
                nwr = [0] * NBANK
                order = [(b, ci) for b in range(BL) for ci in range(3)]
                for b, ci in order:
                    nwr[bank_of(b, ci)] += 1
                seen = [0] * NBANK
                for b, ci in order:
                    c0, cl = NCH[ci]
                    bk = bank_of(b, ci)
                    seen[bk] += 1
                    nc.tensor.matmul(
                        preact[0:fout, b * NB + c0: b * NB + c0 + cl],
                        rbt[ci][0:cl, b * fout:(b + 1) * fout],
                        ident[0:cl, 0:cl],
                        is_transpose=True,
                        start=False, stop=(seen[bk] == nwr[bk]))
                return preact

            def bank_of(b, ci):
                return (b * NB + NCH[ci][0]) // 512

            def cell(cname, xh_t, xh_next):
                """One DCGRU cell. x in xh_t[64:128], h in xh_t[0:64].
                Writes h' to xh_t[0:64] and (if xh_next) xh_next[64:128]."""
                din = CELL_DIN[cname]
                gparts = [(lambda w: w[0:128, :], xh_t[0:128, :], None)]
                pre_g = gconv(cname, gparts, "g", 128, bank_of)
                gb = wt[f"{cname}_gb"]
                for ci in range(NBANK):
                    sl = slice(ci * 512, (ci + 1) * 512)
                    nc.scalar.activation(r_t[:, sl], pre_g[0:64, sl],
                                         AFT.Sigmoid, bias=gb[0:64, 0:1])
                    nc.scalar.activation(u_t[:, sl], pre_g[64:128, sl],
                                         AFT.Sigmoid, bias=gb[64:128, 0:1])
                # xr: rows 0:64 = r*h, rows 64:64+din = x (copy)
                nc.vector.tensor_tensor(xr[0:64, :], r_t[:, :], xh_t[0:64, :],
                                        mybir.AluOpType.mult)
                nc.vector.tensor_copy(xr[64:64 + din, :],
                                      xh_t[64:64 + din, :])
                cparts = [(lambda w: w[0:128, :], xr[0:128, :], None)]
                pre_c = gconv(cname, cparts, "c", 64, bank_of)
                cb = wt[f"{cname}_cb"]
                for ci in range(NBANK):
                    sl = slice(ci * 512, (ci + 1) * 512)
                    nc.scalar.activation(c_t[:, sl], pre_c[0:64, sl],
                                         AFT.Tanh, bias=cb[0:64, 0:1])
                # h' = c + u*(h-c)
                nc.vector.tensor_tensor(t1[:, :], xh_t[0:64, :], c_t[:, :],
                                        mybir.AluOpType.subtract)
                nc.vector.tensor_tensor(t2[:, :], u_t[:, :], t1[:, :],
                                        mybir.AluOpType.mult)
                nc.vector.tensor_tensor(xh_t[0:64, :], c_t[:, :], t2[:, :],
                                        mybir.AluOpType.add)
                if xh_next is not None:
                    nc.vector.tensor_copy(xh_next[64:128, :], xh_t[0:64, :])

            # ---- encoder ----
            for t in range(nsteps_enc):
                nc.sync.dma_start(out=xh["enc0"][64:66, :], in_=x_in[t])
                cell("enc0", xh["enc0"], xh["enc1"])
                cell("enc1", xh["enc1"], None)

            # ---- copy encoder state to decoder ----
            nc.vector.tensor_copy(xh["dec0"][0:64, :], xh["enc0"][0:64, :])
            nc.vector.tensor_copy(xh["dec1"][0:64, :], xh["enc1"][0:64, :])

            # ---- decoder ----
            for t in range(nsteps_dec):
                cell("dec0", xh["dec0"], xh["dec1"])
                cell("dec1", xh["dec1"], None)
                # projection: out = hd1' @ pW + pb -> xh_dec0 row 64 + DRAM
                for ci in range(NBANK):
                    sl = slice(ci * 512, (ci + 1) * 512)
                    pt = psp.tile([128, 512], F32, tag="pstage")
                    nc.tensor.matmul(pt[0:1, :], wt["pW"][0:64, :],
                                     xh["dec1"][0:64, sl],
                                     start=True, stop=True)
                    nc.scalar.activation(xh["dec0"][64:65, sl], pt[0:1, :],
                                         AFT.Identity, bias=wt["pb"][0:1, 0:1])
                ov = xh["dec0"][64:65, :].bitcast(F32).rearrange("p (b n) -> p b n", b=BL)
                nc.sync.dma_start(out=out_d[t], in_=ov[:, :, 0:N])

    nc.finalize()
    _BUILD_CACHE[key] = nc
    return nc


def _prep_inputs(inputs, support, weights):
    """Host-side prep. Returns (shared_map, per_core_x list)."""
    shared = {"s": np.ascontiguousarray(support, np.float32)}
    for c in CELLS:
        din = CELL_DIN[c]
        ga0, gw1, gw2 = _pad_w(weights[f"{c}_gate_W"], din, 2 * U)
        ca0, cw1, cw2 = _pad_w(weights[f"{c}_cand_W"], din, U)
        gb = np.zeros((128, 1), np.float32)
        gb[:, 0] = weights[f"{c}_gate_b"]
        cb = np.zeros((64, 1), np.float32)
        cb[:, 0] = weights[f"{c}_cand_b"]
        shared.update({f"{c}_gA0": ga0, f"{c}_gW1": gw1, f"{c}_gW2": gw2,
                       f"{c}_cA0": ca0, f"{c}_cW1": cw1, f"{c}_cW2": cw2,
                       f"{c}_gb": gb, f"{c}_cb": cb})
    shared["pW"] = np.ascontiguousarray(weights["proj_W"], np.float32)
    shared["pb"] = np.asarray(weights["proj_b"], np.float32).reshape(1, 1)

    # inputs (T, B, N*DIN) -> per-core (T, DIN, AF) with node padding
    x = np.asarray(inputs, np.float32).reshape(T, B, N, DIN)
    per_core = []
    for c in range(NCORES):
        xc = x[:, c * BL:(c + 1) * BL]                  # (T, BL, N, DIN)
        xp = np.zeros((T, DIN, BL, NB), np.float32)
        xp[:, :, :, 0:N] = xc.transpose(0, 3, 1, 2)
        per_core.append(xp.reshape(T, DIN, AF))
    return shared, per_core


def kernel(**inputs) -> np.ndarray:
    support = np.asarray(inputs["support"], np.float32)
    weights = {k: np.asarray(v, np.float32) for k, v in inputs.items()
               if k not in ("inputs", "support")}
    shared, per_core_x = _prep_inputs(inputs["inputs"], support, weights)

    nc = _build(T, HZ)
    if os.environ.get("DCRNN_TRACE"):
        _install_ntff_hook()
    in_maps = [dict(shared, x=per_core_x[c]) for c in range(NCORES)]
    res = run_bass_kernel_spmd(nc, in_maps, list(range(NCORES)),
                               trace=bool(os.environ.get("DCRNN_TRACE")))
    global LAST_RESULT
    LAST_RESULT = res
    if res.exec_time_ns is not None:
        print(f"HW exec time: {res.exec_time_ns} ns")
    outs = [res.results[c]["out"].reshape(HZ, BL, N) for c in range(NCORES)]
    return np.concatenate(outs, axis=1).astype(np.float32)


if __name__ == "__main__":
    sys.path.insert(0, "/root/problem")
    import reference
    ins = reference.setup_inputs()
    ins = {k: np.asarray(v) for k, v in ins.items()}
    exp = np.asarray(reference.reference(**ins))
    act = kernel(**ins)
    err = np.max(np.abs(act - exp)) / (np.abs(exp).max() + 1e-30)
    print("Relative error:", err)
